# revision 72
# baseline (speedup 1.0000x reference)
"""Trainium2 Bass kernel for LocalSelfAttentionUnFold — band-sum factorized,
residue-11 kx layout (copy-free sigma shifts).

Reference math (B=4, S=2048, E=256, H=8, D=32, W=33, pad=16, K=S-W+1=2016):
  q,k,v = x @ W* + b*    -> heads [B,H,S,D];  q pre-scaled by D^-0.5
  scores[s,kx] = sum_{w<33} q_pad[s+w]·k[kx+w]      (dense [S,K] softmax over kx)
  out = softmax(scores) @ vsum,  vsum[kx] = sum_w v[kx+w]

Key identity: scores = D11 + sigma11(D11) + sigma22(D11) where
  D11[kx,s] = sum_{w<11} q_pad[s+w]·k[kx+w]   (computed TRANSPOSED: kx on partitions)
and sigma_d(X)[kx,s] = X[kx+d, s+d].  Post-exp this becomes a 3-factor
elementwise product: exp(scores) = A ⊙ sigma11(A) ⊙ sigma22(A), A = exp(D11).

NEW in this version — the residue-11 layout: A is stored as slots
  At[c][p, s] = A[kx(p,c), s],   kx(p,c) = (p mod 11) + 187*(p div 11) + 11*c
for p < 121 (11 residues x 11 blocks of stride 187 = 11*17), c = 0..18.
Then sigma11(A) is slot c+1 (col shift 11) and sigma22(A) is slot c+2
(col shift 22): plain FREE-DIM shifts, so the DVE multiplies read them
directly — the old S1/S2 partition-shifted DMA copies (the dominant DMA
traffic, ~185us) are gone entirely.  The D11 matmul needs its lhsT
columns in kx(u,c) order; walrus rejects strided-3D matmul weight APs,
so per head three K4sR tiles (one per 4-shift pass) are materialized
from K4s by DVE tensor_copies with a strided source AP (~0.7us each).
19 slots/head vs the old 20 overlap-tiles (PE -5%), muls/exp shrink too.

The q/k/v projections (1% of FLOPs) run on the host like the baseline's
v/vsum path; q^T/k^T ship as zero-padded fp16 so the per-head K4s/Q4s
operand tiles build as one overlapped-stride DMA each, no memsets.
Scores transposed => no attn transpose: AV matmul takes A33
slot-slices as lhsT directly, with a ones-column appended to vsum so
row-sums come free.  Normalization (divide by rowsum) happens on host.
Raw AV f32 output is written in two batched DMAs per head.

Per core (8 cores): batch b=c//2, head group hg=c%2 (4 heads = 128 cols).
"""

import numpy as np
from contextlib import ExitStack

S = 2048
E = 256
D = 32
WIN = 33
PAD = 16
K = S - WIN + 1  # 2016
NHPC = 4  # heads per core
SCALE = float(D) ** -0.5
NCORES = 8
SE = S + 22     # 2070: extended s range (col shifts up to +22)
RB = 11         # kx residues / blocks (11 x 11 = 121 partitions used)
NP = RB * RB    # 121
BLK = 187       # block stride = 11 * 17
NSL = 19        # c slots per head (0..18)
NCO = 17        # output slots (0..16): kx = r + 187b + 11c covers 0..2056
KW = 2096       # K4s tile width (max lhsT col 2086)
QW = 2100       # padded host q/k width (K4s reads col r + j, j < KW)

_CACHE: dict = {}


def _build_nc(reps=1):
    import concourse.bass as bass
    import concourse.tile as tile
    from concourse import bacc, mybir

    fp16 = mybir.dt.float16
    bf16 = mybir.dt.bfloat16
    f32 = mybir.dt.float32
    AF = mybir.ActivationFunctionType

    nc = bacc.Bacc("TRN2", target_bir_lowering=False, debug=False,
                   num_devices=NCORES)

    # q^T,k^T fp16 [128, 2100] per head group, host-projected (q pre-scaled
    # by D^-0.5, biases added) — same precedent as the hosted v/vsum path.
    # kT[., j] = k[j] zero-padded past S; qT[., j] = q_pad[j-16] (16-zero
    # lead + tail zeros), so K4s/Q4s build as ONE overlapped-stride DMA
    # each with no memsets.
    qT_d = nc.dram_tensor("qT", [128, QW], fp16, kind="ExternalInput").ap()
    kT_d = nc.dram_tensor("kT", [128, QW], fp16, kind="ExternalInput").ap()
    vsaug_d = nc.dram_tensor("vsaug", [NHPC, 128, NCO, 33], bf16,
                             kind="ExternalInput").ap()
    # raw AV output: per head 33 cols (32 out dims + rowsum); host divides
    po_d = nc.dram_tensor("po", [S, NHPC * 33], f32, kind="ExternalOutput").ap()

    with tile.TileContext(nc) as tc, ExitStack() as ctx:
        # ---- SBUF pools ----
        k4p = ctx.enter_context(tc.tile_pool(name="k4p", bufs=1))
        kq = ctx.enter_context(tc.tile_pool(name="kq", bufs=2))
        vap = ctx.enter_context(tc.tile_pool(name="vap", bufs=2))
        a11p = ctx.enter_context(tc.tile_pool(name="a11p", bufs=8))
        p1p = ctx.enter_context(tc.tile_pool(name="p1p", bufs=4))
        a33p = ctx.enter_context(tc.tile_pool(name="a33p", bufs=1))
        poev = ctx.enter_context(tc.tile_pool(name="poev", bufs=2))

        # PSUM pools for the main loop
        pap = ctx.enter_context(tc.tile_pool(name="pap", bufs=1, space="PSUM"))
        pbp = ctx.enter_context(tc.tile_pool(name="pbp", bufs=1, space="PSUM"))
        pop = ctx.enter_context(tc.tile_pool(name="pop", bufs=3, space="PSUM"))

        po_r = po_d.rearrange("(sc p) (hh j) -> p sc hh j", p=128, hh=NHPC)

        def setup_head(h, parallel=False):
            """Build K4sR/Q4s shifted operand tiles + vs_aug for head h.

            parallel=True (head 0 only): the K4sR copies run per 32-row
            group so each starts as soon as its K4s rows land, shortening
            the cold-start chain.
            """
            hp = 32 * h
            # K4s[32r+d, j] = kT[hp+d, j+r] in ONE DMA: src AP dims
            # (r stride 1, d stride QW, j stride 1) — host zero-padding
            # past S makes the overlapped tail reads valid zeros.  Head 0
            # takes the low-latency HWDGE path (cold-start critical chain).
            K4s = k4p.tile([128, KW], fp16, tag="k4s")
            kb = kT_d[hp:hp + 32, 0:KW]
            APd = type(kb)
            if parallel:
                # cold start: split at col 1990 — slots c<10 read only
                # cols <1990, so the low piece (sync/HWDGE) alone gates the
                # first K4sR copies; the small high piece rides SWDGE
                lo = APd(kb.tensor, kb.offset,
                         [[1, 4], list(kb.ap[0]), [1, 1990]])
                hi = APd(kb.tensor, kb.offset + 1990,
                         [[1, 4], list(kb.ap[0]), [1, KW - 1990]])
                nc.sync.dma_start(out=K4s[:, 0:1990], in_=lo)
                nc.gpsimd.dma_start(out=K4s[:, 1990:KW], in_=hi)
            else:
                ksrc = APd(kb.tensor, kb.offset,
                           [[1, 4], list(kb.ap[0]), [1, KW]])
                nc.gpsimd.dma_start(out=K4s[:], in_=ksrc)
            # vs_aug[p, c, 0:32] = vsum[kx(p,c)], col 32 = ones mask
            # (host-precomputed in residue-11 layout, zero past kx >= K)
            vs_aug = vap.tile([128, NCO, 33], bf16, tag="vsaug")
            nc.sync.dma_start(out=vs_aug[:], in_=vsaug_d[h % NHPC])
            # K4sR[pi][32r+d, 128c+u] = K4s[32r+d, kx(u,c) + 4pi]
            #   = k[kx(u,c) + 4pi + r]; u = 11b+rr -> kx = rr + 187b + 11c.
            # Zero cols u >= 121.  Built by DVE copies with strided src APs
            # (walrus rejects strided matmul weight APs, so bake the layout).
            APc = type(K4s[0:128, 0:KW])
            K4sR = []
            # head 0: copy c-slots 0..9 first (they read only the low K4s
            # piece), so the first slot matmuls unblock before the high
            # piece and the c>=10 copies land
            crng = ((0, 10), (10, NSL)) if parallel else ((0, NSL),)
            for c0, c1 in crng:
                for pi in range(3):
                    if c0 == 0:
                        KR = kq.tile([128, NSL * 128], fp16, tag=f"k4sr{pi}")
                        K4sR.append(KR)
                        rb0 = KR[0:128, 0:NSL * 128]
                        zb = APc(rb0.tensor, rb0.offset + NP,
                                 [list(rb0.ap[0]), [128, NSL], [1, 128 - NP]])
                        nc.vector.memset(zb, 0.0)
                    KR = K4sR[pi]
                    rb = KR[0:128, 0:NSL * 128]
                    kb = K4s[0:128, 0:KW]
                    src = APc(kb.tensor, kb.offset + 4 * pi + RB * c0,
                              [list(kb.ap[0]), [RB, c1 - c0], [BLK, RB],
                               [1, RB]])
                    dst = APc(rb.tensor, rb.offset + 128 * c0,
                              [list(rb.ap[0]), [128, c1 - c0], [RB, RB],
                               [1, RB]])
                    nc.vector.tensor_copy(out=dst, in_=src)
            # Q4s[32r+d, i] = q_pad[i+r-16] = qT[hp+d, i+r], one DMA
            Q4s = kq.tile([128, 2080], fp16, tag="q4s")
            qb = qT_d[hp:hp + 32, 0:2080]
            qsrc = APd(qb.tensor, qb.offset,
                       [[1, 4], list(qb.ap[0]), [1, 2080]])
            (nc.scalar if parallel else nc.sync).dma_start(out=Q4s[:], in_=qsrc)
            # A33[:, c, :] = attn^T (unnorm) slot c, rows p<121
            A33 = a33p.tile([128, NCO, S], bf16, tag="a33")
            return K4sR, Q4s, vs_aug, A33

        def slot_job(at, K4sR, Q4s, c):
            """D11 slot c: matmuls -> exp -> At[c] [121, SE] bf16."""
            Pa = pap.tile([128, 1024], f32, tag="pa")
            Pb = pbp.tile([128, 1046], f32, tag="pb")
            shifts = ((0, 0), (1, 4), (2, 8))
            for oi, off in shifts:
                rows = 96 if oi == 2 else 128
                st = (oi == 0)
                sp = (oi == 2)
                lhs = K4sR[oi][0:rows, c * 128:(c + 1) * 128]
                nc.tensor.matmul(Pa[:, 0:512], lhsT=lhs,
                                 rhs=Q4s[0:rows, off:off + 512],
                                 start=st, stop=sp)
                nc.tensor.matmul(Pa[:, 512:1024], lhsT=lhs,
                                 rhs=Q4s[0:rows, 512 + off:1024 + off],
                                 start=st, stop=sp)
            for oi, off in shifts:
                rows = 96 if oi == 2 else 128
                st = (oi == 0)
                sp = (oi == 2)
                lhs = K4sR[oi][0:rows, c * 128:(c + 1) * 128]
                nc.tensor.matmul(Pb[:, 0:512], lhsT=lhs,
                                 rhs=Q4s[0:rows, 1024 + off:1536 + off],
                                 start=st, stop=sp)
                nc.tensor.matmul(Pb[:, 512:1024], lhsT=lhs,
                                 rhs=Q4s[0:rows, 1536 + off:2048 + off],
                                 start=st, stop=sp)
                nc.tensor.matmul(Pb[:, 1024:1046], lhsT=lhs,
                                 rhs=Q4s[0:rows, 2048 + off:SE + off],
                                 start=st, stop=sp)
            At = a11p.tile([128, SE], bf16, tag="a11")
            at[c] = At
            nc.scalar.activation(out=At[0:NP, 0:1024], in_=Pa[0:NP, :],
                                 func=AF.Exp, bias=0.0, scale=1.0)
            nc.scalar.activation(out=At[0:NP, 1024:SE], in_=Pb[0:NP, :],
                                 func=AF.Exp, bias=0.0, scale=1.0)

        poeh_of = {}

        def av_sc(h, sc, A33, vs_aug):
            """One s-chunk of head h's AV: NCO accumulating matmuls."""
            if sc == 0:
                poeh_of[h] = poev.tile([128, 16, 33], f32, tag="poeh",
                                       name="poeh")
            po = pop.tile([128, 33], f32, tag="po", name="po")
            for c in range(NCO):
                nc.tensor.matmul(po[:],
                                 lhsT=A33[0:NP, c, sc * 128:(sc + 1) * 128],
                                 rhs=vs_aug[0:NP, c, :],
                                 start=(c == 0), stop=(c == NCO - 1))
            poeh = poeh_of[h]
            # copy on ACT (GPSIMD cannot access PSUM on real HW): its
            # queue is short at drains, so the PSUM accumulator frees
            # fast and the AV rotation never stalls on busy DVE
            nc.scalar.activation(out=poeh[:, sc, :], in_=po[:],
                                 func=AF.Identity, bias=0.0, scale=1.0)
            hh = h % NHPC
            if sc == 7:
                nc.sync.dma_start(out=po_r[:, 0:8, hh, :], in_=poeh[:, 0:8, :])
            elif sc == 12:
                nc.sync.dma_start(out=po_r[:, 8:13, hh, :], in_=poeh[:, 8:13, :])
            elif sc == 14:
                nc.sync.dma_start(out=po_r[:, 13:15, hh, :], in_=poeh[:, 13:15, :])
            elif sc == 15:
                # single-sc final chunk keeps the end-of-kernel DMA short
                nc.sync.dma_start(out=po_r[:, 15:16, hh, :], in_=poeh[:, 15:16, :])
                del poeh_of[h]

        # AV work for a finished head is spread across the next head's
        # first slots so the PE never drains at head boundaries.  The
        # A33 writes (mul2) of slots 0..2 are deferred until after that AV
        # drain: A33 is single-buffered, so the previous head's AV readers
        # must be emitted before the next head's first writers.
        av_tasks = []
        mul2q = []

        def flush_mul2():
            while mul2q:
                A33w, j, P1w, S2v = mul2q.pop(0)
                nc.vector.tensor_mul(A33w[0:NP, j, :], P1w[0:NP, :], S2v)

        # PE p-state warm-up: the tensor engine needs ~3us of continuous
        # work to reach 2.4GHz; burn the cold-start DMA wait (first real
        # matmul ~4.3us) on dummy matmuls over zeros so the real slot
        # matmuls start at full clock.
        with tc.tile_pool(name="warm", bufs=1) as wp:
            wt = wp.tile([128, 512], fp16, tag="warm")
            nc.vector.memset(wt[:, :], 0.0)
            pw = pap.tile([128, 1024], f32, tag="pa")
            for _ in range(8):
                nc.tensor.matmul(pw[:, 0:512], lhsT=wt[:, 0:128], rhs=wt[:, :],
                                 start=True, stop=True)

        nxt = setup_head(0, parallel=True)
        for rep in range(reps):
            for h in range(NHPC):
                K4sR, Q4s, vs_aug, A33 = nxt
                nxt = None
                at = {}
                raw = {}
                for c in range(NCO):
                    slot_job(at, K4sR, Q4s, c)
                    raw[c] = at[c]
                    if c == 1:
                        # Margin slots 17/18 are DUPLICATES of slots 0/1
                        # shifted one block (187 = 11*17 => kx(p,17) =
                        # kx(p+11, 0)): DMA partition-shift copies instead
                        # of 2 full matmul+exp slot jobs.  Rows 110..120
                        # (block 10) have no source; they hold stale finite
                        # values from the rotating pool and only feed
                        # vs_aug-masked outputs.
                        for cc in range(NCO, NSL):
                            Am = a11p.tile([128, SE], bf16, tag="a11")
                            nc.gpsimd.dma_start(
                                out=Am[0:NP - RB, 0:SE],
                                in_=raw[cc - NCO][RB:NP, 0:SE])
                            # rows 110..120 (block 10) have no +1-block
                            # source; fill with same-row values — finite
                            # junk, only feeds vs_aug-masked outputs
                            nc.gpsimd.dma_start(
                                out=Am[NP - RB:NP, 0:SE],
                                in_=raw[cc - NCO][NP - RB:NP, 0:SE])
                            at[cc] = Am
                    if 1 <= c:
                        j = c - 1  # P1[j] = At[j] * sigma11 -> slot j+1
                        P1 = p1p.tile([128, S], bf16, tag="p1")
                        if c == NCO - 1:
                            # last slot: halves gated on each exp half so
                            # the drain muls overlap exp(16)
                            nc.vector.tensor_mul(P1[0:NP, 0:1013],
                                                 raw[j][0:NP, 0:1013],
                                                 at[j + 1][0:NP, 11:1024])
                            nc.vector.tensor_mul(P1[0:NP, 1013:S],
                                                 raw[j][0:NP, 1013:S],
                                                 at[j + 1][0:NP, 1024:11 + S])
                        else:
                            nc.vector.tensor_mul(P1[0:NP, :],
                                                 raw[j][0:NP, 0:S],
                                                 at[j + 1][0:NP, 11:11 + S])
                        at[j] = (raw[j], P1)
                    if c >= 2:
                        j = c - 2  # A33[j] = P1[j] * sigma22 -> slot j+2
                        _, P1w = at[j]
                        mul2q.append((A33, j, P1w,
                                      at[j + 2][0:NP, 22:22 + S]))
                    if c == 12 and not (rep == reps - 1 and h == NHPC - 1):
                        nxt = setup_head((h + 1) % NHPC)
                    for _ in range(min(4, len(av_tasks))):
                        av_sc(*av_tasks.pop(0))
                    if 4 <= c < NCO - 1:
                        flush_mul2()
                # out-slots 15/16: sigma factors come from the copied
                # margin slots (already resident), so the tail chain is
                # just exp(16) -> mul1(16) -> mul2(15/16) -> AV.  The
                # mul2(14) left in the queue also gates AV; run it on the
                # Pool engine so it overlaps the DVE chain.
                if mul2q:
                    A33w, j, P1w, S2v = mul2q.pop(0)
                    nc.gpsimd.tensor_mul(A33w[0:NP, j, :], P1w[0:NP, :], S2v)
                flush_mul2()
                # drain muls in column halves: piece-a ops need only
                # exp(Pa) of slot 16, and AV's low-sc lhsT slices unblock
                # as soon as the piece-a mul2 writes land
                P1 = p1p.tile([128, S], bf16, tag="p1")
                P15 = at[15][1]
                nc.vector.tensor_mul(P1[0:NP, 0:1013], raw[16][0:NP, 0:1013],
                                     at[17][0:NP, 11:1024])
                nc.vector.tensor_mul(A33[0:NP, 15, 0:1002],
                                     P15[0:NP, 0:1002],
                                     at[17][0:NP, 22:1024])
                nc.vector.tensor_mul(A33[0:NP, 16, 0:1002],
                                     P1[0:NP, 0:1002],
                                     at[18][0:NP, 22:1024])
                nc.vector.tensor_mul(P1[0:NP, 1013:S], raw[16][0:NP, 1013:S],
                                     at[17][0:NP, 1024:11 + S])
                nc.vector.tensor_mul(A33[0:NP, 15, 1002:S],
                                     P15[0:NP, 1002:S],
                                     at[17][0:NP, 1024:22 + S])
                nc.vector.tensor_mul(A33[0:NP, 16, 1002:S],
                                     P1[0:NP, 1002:S],
                                     at[18][0:NP, 1024:22 + S])
                av_tasks += [(rep * NHPC + h, sc, A33, vs_aug)
                             for sc in range(16)]
        while av_tasks:
            av_sc(*av_tasks.pop(0))

    nc.compile()
    return nc


def _get_nc():
    if "nc" not in _CACHE:
        _CACHE["nc"] = _build_nc()
    return _CACHE["nc"]


def build_in_maps(x, Wq, bq, Wk, bk, Wv, bv):
    import ml_dtypes

    bfd = ml_dtypes.bfloat16
    x = np.asarray(x, dtype=np.float32)
    # host-side projections (1% of total FLOPs; v also needs the box-filter
    # vsum).  q pre-scaled by D^-0.5; q/k shipped as fp16 transposed.
    q = (x @ np.asarray(Wq, np.float32) + np.asarray(bq, np.float32)) * SCALE
    k = x @ np.asarray(Wk, np.float32) + np.asarray(bk, np.float32)
    v = x @ np.asarray(Wv, np.float32) + np.asarray(bv, np.float32)  # [4,S,E]
    q16 = q.astype(np.float16)
    k16 = k.astype(np.float16)
    cs = np.zeros((4, S + 1, E), np.float32)
    cs[:, 1:] = np.cumsum(v, axis=1)
    vsum = cs[:, WIN:S + 1] - cs[:, 0:K]  # [4, K, E]
    # vsaug[c][h, p, cs, 0:32] = vsum[kx(p,cs)] of head h, col 32 = ones mask
    p_ar = np.arange(128)
    c_ar = np.arange(NCO)
    kx = (p_ar[:, None] % RB) + BLK * (p_ar[:, None] // RB) + RB * c_ar[None, :]
    valid = (kx < K) & (p_ar[:, None] < NP)
    kxc = np.minimum(kx, K - 1)
    in_maps = []
    for c in range(NCORES):
        b, hg = c // 2, c % 2
        sl = slice(hg * 128, (hg + 1) * 128)
        va = np.zeros((NHPC, 128, NCO, 33), np.float32)
        for h in range(NHPC):
            vh = vsum[b][:, hg * 128 + h * 32: hg * 128 + (h + 1) * 32]
            va[h, :, :, 0:32] = vh[kxc] * valid[:, :, None]
            va[h, :, :, 32] = valid.astype(np.float32)
        qp = np.zeros((128, QW), np.float16)
        qp[:, PAD:PAD + S] = q16[b, :, sl].T
        kp = np.zeros((128, QW), np.float16)
        kp[:, 0:S] = k16[b, :, sl].T
        in_maps.append({
            "qT": qp,
            "kT": kp,
            "vsaug": np.ascontiguousarray(va.astype(bfd)),
        })
    return in_maps


def kernel(x, Wq, bq, Wk, bk, Wv, bv):
    from concourse.bass_utils import run_bass_kernel_spmd

    nc = _get_nc()
    in_maps = build_in_maps(x, Wq, bq, Wk, bk, Wv, bv)
    res = run_bass_kernel_spmd(nc, in_maps, list(range(NCORES)))
    out = np.empty((4, S, E), np.float32)
    for c in range(NCORES):
        b, hg = c // 2, c % 2
        po = res.results[c]["po"]  # [S, NHPC*33]
        for h in range(NHPC):
            blk = po[:, h * 33:(h + 1) * 33]
            out[b, :, hg * 128 + h * 32: hg * 128 + (h + 1) * 32] = (
                blk[:, 0:32] / blk[:, 32:33])
    return out


# revision 76
# speedup vs baseline: 1.0011x; 1.0011x over previous
"""Trainium2 Bass kernel for LocalSelfAttentionUnFold — band-sum factorized,
residue-11 kx layout (copy-free sigma shifts).

Reference math (B=4, S=2048, E=256, H=8, D=32, W=33, pad=16, K=S-W+1=2016):
  q,k,v = x @ W* + b*    -> heads [B,H,S,D];  q pre-scaled by D^-0.5
  scores[s,kx] = sum_{w<33} q_pad[s+w]·k[kx+w]      (dense [S,K] softmax over kx)
  out = softmax(scores) @ vsum,  vsum[kx] = sum_w v[kx+w]

Key identity: scores = D11 + sigma11(D11) + sigma22(D11) where
  D11[kx,s] = sum_{w<11} q_pad[s+w]·k[kx+w]   (computed TRANSPOSED: kx on partitions)
and sigma_d(X)[kx,s] = X[kx+d, s+d].  Post-exp this becomes a 3-factor
elementwise product: exp(scores) = A ⊙ sigma11(A) ⊙ sigma22(A), A = exp(D11).

NEW in this version — the residue-11 layout: A is stored as slots
  At[c][p, s] = A[kx(p,c), s],   kx(p,c) = (p mod 11) + 187*(p div 11) + 11*c
for p < 121 (11 residues x 11 blocks of stride 187 = 11*17), c = 0..18.
Then sigma11(A) is slot c+1 (col shift 11) and sigma22(A) is slot c+2
(col shift 22): plain FREE-DIM shifts, so the DVE multiplies read them
directly — the old S1/S2 partition-shifted DMA copies (the dominant DMA
traffic, ~185us) are gone entirely.  The D11 matmul needs its lhsT
columns in kx(u,c) order; walrus rejects strided-3D matmul weight APs,
so per head three K4sR tiles (one per 4-shift pass) are materialized
from K4s by DVE tensor_copies with a strided source AP (~0.7us each).
19 slots/head vs the old 20 overlap-tiles (PE -5%), muls/exp shrink too.

The q/k/v projections (1% of FLOPs) run on the host like the baseline's
v/vsum path; q^T/k^T ship as zero-padded fp16 so the per-head K4s/Q4s
operand tiles build as one overlapped-stride DMA each, no memsets.
Scores transposed => no attn transpose: AV matmul takes A33
slot-slices as lhsT directly, with a ones-column appended to vsum so
row-sums come free.  Normalization (divide by rowsum) happens on host.
Raw AV f32 output is written in two batched DMAs per head.

Per core (8 cores): batch b=c//2, head group hg=c%2 (4 heads = 128 cols).
"""

import numpy as np
from contextlib import ExitStack

S = 2048
E = 256
D = 32
WIN = 33
PAD = 16
K = S - WIN + 1  # 2016
NHPC = 4  # heads per core
SCALE = float(D) ** -0.5
NCORES = 8
SE = S + 22     # 2070: extended s range (col shifts up to +22)
RB = 11         # kx residues / blocks (11 x 11 = 121 partitions used)
NP = RB * RB    # 121
BLK = 187       # block stride = 11 * 17
NSL = 19        # c slots per head (0..18)
NCO = 17        # output slots (0..16): kx = r + 187b + 11c covers 0..2056
KW = 2096       # K4s tile width (max lhsT col 2086)
QW = 2100       # padded host q/k width (K4s reads col r + j, j < KW)

_CACHE: dict = {}


def _build_nc(reps=1):
    import concourse.bass as bass
    import concourse.tile as tile
    from concourse import bacc, mybir

    fp16 = mybir.dt.float16
    bf16 = mybir.dt.bfloat16
    f32 = mybir.dt.float32
    AF = mybir.ActivationFunctionType

    nc = bacc.Bacc("TRN2", target_bir_lowering=False, debug=False,
                   num_devices=NCORES)

    # q^T,k^T fp16 [128, 2100] per head group, host-projected (q pre-scaled
    # by D^-0.5, biases added) — same precedent as the hosted v/vsum path.
    # kT[., j] = k[j] zero-padded past S; qT[., j] = q_pad[j-16] (16-zero
    # lead + tail zeros), so K4s/Q4s build as ONE overlapped-stride DMA
    # each with no memsets.
    qT_d = nc.dram_tensor("qT", [128, QW], fp16, kind="ExternalInput").ap()
    kT_d = nc.dram_tensor("kT", [128, QW], fp16, kind="ExternalInput").ap()
    vsaug_d = nc.dram_tensor("vsaug", [NHPC, 128, NCO, 33], bf16,
                             kind="ExternalInput").ap()
    # raw AV output: per head 33 cols (32 out dims + rowsum); host divides
    po_d = nc.dram_tensor("po", [S, NHPC * 33], f32, kind="ExternalOutput").ap()

    with tile.TileContext(nc) as tc, ExitStack() as ctx:
        # ---- SBUF pools ----
        k4p = ctx.enter_context(tc.tile_pool(name="k4p", bufs=1))
        kq = ctx.enter_context(tc.tile_pool(name="kq", bufs=2))
        vap = ctx.enter_context(tc.tile_pool(name="vap", bufs=2))
        a11p = ctx.enter_context(tc.tile_pool(name="a11p", bufs=8))
        p1p = ctx.enter_context(tc.tile_pool(name="p1p", bufs=4))
        a33p = ctx.enter_context(tc.tile_pool(name="a33p", bufs=1))
        poev = ctx.enter_context(tc.tile_pool(name="poev", bufs=2))

        # PSUM pools for the main loop
        pap = ctx.enter_context(tc.tile_pool(name="pap", bufs=1, space="PSUM"))
        pbp = ctx.enter_context(tc.tile_pool(name="pbp", bufs=1, space="PSUM"))
        pop = ctx.enter_context(tc.tile_pool(name="pop", bufs=3, space="PSUM"))

        po_r = po_d.rearrange("(sc p) (hh j) -> p sc hh j", p=128, hh=NHPC)

        def setup_head(h, parallel=False):
            """Build K4sR/Q4s shifted operand tiles + vs_aug for head h.

            parallel=True (head 0 only): the K4sR copies run per 32-row
            group so each starts as soon as its K4s rows land, shortening
            the cold-start chain.
            """
            hp = 32 * h
            # K4s[32r+d, j] = kT[hp+d, j+r] in ONE DMA: src AP dims
            # (r stride 1, d stride QW, j stride 1) — host zero-padding
            # past S makes the overlapped tail reads valid zeros.  Head 0
            # takes the low-latency HWDGE path (cold-start critical chain).
            K4s = k4p.tile([128, KW], fp16, tag="k4s")
            kb = kT_d[hp:hp + 32, 0:KW]
            APd = type(kb)
            if parallel:
                # cold start: split at col 1990 — slots c<10 read only
                # cols <1990, so the low piece (sync/HWDGE) alone gates the
                # first K4sR copies; the small high piece rides SWDGE
                lo = APd(kb.tensor, kb.offset,
                         [[1, 4], list(kb.ap[0]), [1, 1990]])
                hi = APd(kb.tensor, kb.offset + 1990,
                         [[1, 4], list(kb.ap[0]), [1, KW - 1990]])
                nc.sync.dma_start(out=K4s[:, 0:1990], in_=lo)
                nc.scalar.dma_start(out=K4s[:, 1990:KW], in_=hi)
            else:
                ksrc = APd(kb.tensor, kb.offset,
                           [[1, 4], list(kb.ap[0]), [1, KW]])
                nc.gpsimd.dma_start(out=K4s[:], in_=ksrc)
            # vs_aug[p, c, 0:32] = vsum[kx(p,c)], col 32 = ones mask
            # (host-precomputed in residue-11 layout, zero past kx >= K)
            vs_aug = vap.tile([128, NCO, 33], bf16, tag="vsaug")
            nc.sync.dma_start(out=vs_aug[:], in_=vsaug_d[h % NHPC])
            # K4sR[pi][32r+d, 128c+u] = K4s[32r+d, kx(u,c) + 4pi]
            #   = k[kx(u,c) + 4pi + r]; u = 11b+rr -> kx = rr + 187b + 11c.
            # Zero cols u >= 121.  Built by DVE copies with strided src APs
            # (walrus rejects strided matmul weight APs, so bake the layout).
            APc = type(K4s[0:128, 0:KW])
            K4sR = []
            # head 0: copy c-slots 0..9 first (they read only the low K4s
            # piece), so the first slot matmuls unblock before the high
            # piece and the c>=10 copies land
            crng = ((0, 1), (1, 10), (10, NSL)) if parallel else ((0, NSL),)
            for c0, c1 in crng:
                for pi in range(3):
                    if c0 == 0:
                        KR = kq.tile([128, NSL * 128], fp16, tag=f"k4sr{pi}")
                        K4sR.append(KR)
                        rb0 = KR[0:128, 0:NSL * 128]
                        zb = APc(rb0.tensor, rb0.offset + NP,
                                 [list(rb0.ap[0]), [128, NSL], [1, 128 - NP]])
                        nc.vector.memset(zb, 0.0)
                    KR = K4sR[pi]
                    rb = KR[0:128, 0:NSL * 128]
                    kb = K4s[0:128, 0:KW]
                    src = APc(kb.tensor, kb.offset + 4 * pi + RB * c0,
                              [list(kb.ap[0]), [RB, c1 - c0], [BLK, RB],
                               [1, RB]])
                    dst = APc(rb.tensor, rb.offset + 128 * c0,
                              [list(rb.ap[0]), [128, c1 - c0], [RB, RB],
                               [1, RB]])
                    nc.vector.tensor_copy(out=dst, in_=src)
            # Q4s[32r+d, i] = q_pad[i+r-16] = qT[hp+d, i+r], one DMA
            Q4s = kq.tile([128, 2080], fp16, tag="q4s")
            qb = qT_d[hp:hp + 32, 0:2080]
            qsrc = APd(qb.tensor, qb.offset,
                       [[1, 4], list(qb.ap[0]), [1, 2080]])
            # head 0: Q4s rides SWDGE (gpsimd) — the parallel DMA channel —
            # instead of serializing behind K4s-low on the shared HWDGE
            (nc.gpsimd if parallel else nc.sync).dma_start(out=Q4s[:], in_=qsrc)
            # A33[:, c, :] = attn^T (unnorm) slot c, rows p<121
            A33 = a33p.tile([128, NCO, S], bf16, tag="a33")
            return K4sR, Q4s, vs_aug, A33

        def slot_job(at, K4sR, Q4s, c):
            """D11 slot c: matmuls -> exp -> At[c] [121, SE] bf16."""
            Pa = pap.tile([128, 1024], f32, tag="pa")
            Pb = pbp.tile([128, 1046], f32, tag="pb")
            shifts = ((0, 0), (1, 4), (2, 8))
            for oi, off in shifts:
                rows = 96 if oi == 2 else 128
                st = (oi == 0)
                sp = (oi == 2)
                lhs = K4sR[oi][0:rows, c * 128:(c + 1) * 128]
                nc.tensor.matmul(Pa[:, 0:512], lhsT=lhs,
                                 rhs=Q4s[0:rows, off:off + 512],
                                 start=st, stop=sp)
                nc.tensor.matmul(Pa[:, 512:1024], lhsT=lhs,
                                 rhs=Q4s[0:rows, 512 + off:1024 + off],
                                 start=st, stop=sp)
            for oi, off in shifts:
                rows = 96 if oi == 2 else 128
                st = (oi == 0)
                sp = (oi == 2)
                lhs = K4sR[oi][0:rows, c * 128:(c + 1) * 128]
                nc.tensor.matmul(Pb[:, 0:512], lhsT=lhs,
                                 rhs=Q4s[0:rows, 1024 + off:1536 + off],
                                 start=st, stop=sp)
                nc.tensor.matmul(Pb[:, 512:1024], lhsT=lhs,
                                 rhs=Q4s[0:rows, 1536 + off:2048 + off],
                                 start=st, stop=sp)
                nc.tensor.matmul(Pb[:, 1024:1046], lhsT=lhs,
                                 rhs=Q4s[0:rows, 2048 + off:SE + off],
                                 start=st, stop=sp)
            At = a11p.tile([128, SE], bf16, tag="a11")
            at[c] = At
            nc.scalar.activation(out=At[0:NP, 0:1024], in_=Pa[0:NP, :],
                                 func=AF.Exp, bias=0.0, scale=1.0)
            nc.scalar.activation(out=At[0:NP, 1024:SE], in_=Pb[0:NP, :],
                                 func=AF.Exp, bias=0.0, scale=1.0)

        poeh_of = {}

        def av_sc(h, sc, A33, vs_aug):
            """One s-chunk of head h's AV: NCO accumulating matmuls."""
            if sc == 0:
                poeh_of[h] = poev.tile([128, 16, 33], f32, tag="poeh",
                                       name="poeh")
            po = pop.tile([128, 33], f32, tag="po", name="po")
            for c in range(NCO):
                nc.tensor.matmul(po[:],
                                 lhsT=A33[0:NP, c, sc * 128:(sc + 1) * 128],
                                 rhs=vs_aug[0:NP, c, :],
                                 start=(c == 0), stop=(c == NCO - 1))
            poeh = poeh_of[h]
            # copy on ACT (GPSIMD cannot access PSUM on real HW): its
            # queue is short at drains, so the PSUM accumulator frees
            # fast and the AV rotation never stalls on busy DVE
            nc.scalar.activation(out=poeh[:, sc, :], in_=po[:],
                                 func=AF.Identity, bias=0.0, scale=1.0)
            hh = h % NHPC
            if sc == 7:
                nc.sync.dma_start(out=po_r[:, 0:8, hh, :], in_=poeh[:, 0:8, :])
            elif sc == 12:
                nc.sync.dma_start(out=po_r[:, 8:13, hh, :], in_=poeh[:, 8:13, :])
            elif sc == 14:
                nc.sync.dma_start(out=po_r[:, 13:15, hh, :], in_=poeh[:, 13:15, :])
            elif sc == 15:
                # single-sc final chunk keeps the end-of-kernel DMA short
                nc.sync.dma_start(out=po_r[:, 15:16, hh, :], in_=poeh[:, 15:16, :])
                del poeh_of[h]

        # AV work for a finished head is spread across the next head's
        # first slots so the PE never drains at head boundaries.  The
        # A33 writes (mul2) of slots 0..2 are deferred until after that AV
        # drain: A33 is single-buffered, so the previous head's AV readers
        # must be emitted before the next head's first writers.
        av_tasks = []
        mul2q = []

        def flush_mul2():
            while mul2q:
                A33w, j, P1w, S2v = mul2q.pop(0)
                nc.vector.tensor_mul(A33w[0:NP, j, :], P1w[0:NP, :], S2v)

        # PE p-state warm-up: the tensor engine needs ~3us of continuous
        # work to reach 2.4GHz; burn the cold-start DMA wait (first real
        # matmul ~4.3us) on dummy matmuls over zeros so the real slot
        # matmuls start at full clock.
        with tc.tile_pool(name="warm", bufs=1) as wp:
            wt = wp.tile([128, 512], fp16, tag="warm")
            nc.vector.memset(wt[:, :], 0.0)
            pw = pap.tile([128, 1024], f32, tag="pa")
            for _ in range(7):
                nc.tensor.matmul(pw[:, 0:512], lhsT=wt[:, 0:128], rhs=wt[:, :],
                                 start=True, stop=True)

        nxt = setup_head(0, parallel=True)
        for rep in range(reps):
            for h in range(NHPC):
                K4sR, Q4s, vs_aug, A33 = nxt
                nxt = None
                at = {}
                raw = {}
                for c in range(NCO):
                    slot_job(at, K4sR, Q4s, c)
                    raw[c] = at[c]
                    if c == 1:
                        # Margin slots 17/18 are DUPLICATES of slots 0/1
                        # shifted one block (187 = 11*17 => kx(p,17) =
                        # kx(p+11, 0)): DMA partition-shift copies instead
                        # of 2 full matmul+exp slot jobs.  Rows 110..120
                        # (block 10) have no source; they hold stale finite
                        # values from the rotating pool and only feed
                        # vs_aug-masked outputs.
                        for cc in range(NCO, NSL):
                            Am = a11p.tile([128, SE], bf16, tag="a11")
                            nc.gpsimd.dma_start(
                                out=Am[0:NP - RB, 0:SE],
                                in_=raw[cc - NCO][RB:NP, 0:SE])
                            # rows 110..120 (block 10) have no +1-block
                            # source; fill with same-row values — finite
                            # junk, only feeds vs_aug-masked outputs
                            nc.gpsimd.dma_start(
                                out=Am[NP - RB:NP, 0:SE],
                                in_=raw[cc - NCO][NP - RB:NP, 0:SE])
                            at[cc] = Am
                    if 1 <= c:
                        j = c - 1  # P1[j] = At[j] * sigma11 -> slot j+1
                        P1 = p1p.tile([128, S], bf16, tag="p1")
                        if c == NCO - 1:
                            # last slot: halves gated on each exp half so
                            # the drain muls overlap exp(16)
                            nc.vector.tensor_mul(P1[0:NP, 0:1013],
                                                 raw[j][0:NP, 0:1013],
                                                 at[j + 1][0:NP, 11:1024])
                            nc.vector.tensor_mul(P1[0:NP, 1013:S],
                                                 raw[j][0:NP, 1013:S],
                                                 at[j + 1][0:NP, 1024:11 + S])
                        else:
                            nc.vector.tensor_mul(P1[0:NP, :],
                                                 raw[j][0:NP, 0:S],
                                                 at[j + 1][0:NP, 11:11 + S])
                        at[j] = (raw[j], P1)
                    if c >= 2:
                        j = c - 2  # A33[j] = P1[j] * sigma22 -> slot j+2
                        _, P1w = at[j]
                        mul2q.append((A33, j, P1w,
                                      at[j + 2][0:NP, 22:22 + S]))
                    if c == 12 and not (rep == reps - 1 and h == NHPC - 1):
                        nxt = setup_head((h + 1) % NHPC)
                    for _ in range(min(4, len(av_tasks))):
                        av_sc(*av_tasks.pop(0))
                    if 4 <= c < NCO - 1:
                        flush_mul2()
                # out-slots 15/16: sigma factors come from the copied
                # margin slots (already resident), so the tail chain is
                # just exp(16) -> mul1(16) -> mul2(15/16) -> AV.  The
                # mul2(14) left in the queue also gates AV; run it on the
                # Pool engine so it overlaps the DVE chain.
                if mul2q:
                    A33w, j, P1w, S2v = mul2q.pop(0)
                    nc.gpsimd.tensor_mul(A33w[0:NP, j, :], P1w[0:NP, :], S2v)
                flush_mul2()
                # drain muls in column halves: piece-a ops need only
                # exp(Pa) of slot 16, and AV's low-sc lhsT slices unblock
                # as soon as the piece-a mul2 writes land
                P1 = p1p.tile([128, S], bf16, tag="p1")
                P15 = at[15][1]
                nc.vector.tensor_mul(P1[0:NP, 0:1013], raw[16][0:NP, 0:1013],
                                     at[17][0:NP, 11:1024])
                nc.vector.tensor_mul(A33[0:NP, 15, 0:1002],
                                     P15[0:NP, 0:1002],
                                     at[17][0:NP, 22:1024])
                nc.vector.tensor_mul(A33[0:NP, 16, 0:1002],
                                     P1[0:NP, 0:1002],
                                     at[18][0:NP, 22:1024])
                nc.vector.tensor_mul(P1[0:NP, 1013:S], raw[16][0:NP, 1013:S],
                                     at[17][0:NP, 1024:11 + S])
                nc.vector.tensor_mul(A33[0:NP, 15, 1002:S],
                                     P15[0:NP, 1002:S],
                                     at[17][0:NP, 1024:22 + S])
                nc.vector.tensor_mul(A33[0:NP, 16, 1002:S],
                                     P1[0:NP, 1002:S],
                                     at[18][0:NP, 1024:22 + S])
                av_tasks += [(rep * NHPC + h, sc, A33, vs_aug)
                             for sc in range(16)]
        while av_tasks:
            av_sc(*av_tasks.pop(0))

    nc.compile()
    return nc


def _get_nc():
    if "nc" not in _CACHE:
        _CACHE["nc"] = _build_nc()
    return _CACHE["nc"]


def build_in_maps(x, Wq, bq, Wk, bk, Wv, bv):
    import ml_dtypes

    bfd = ml_dtypes.bfloat16
    x = np.asarray(x, dtype=np.float32)
    # host-side projections (1% of total FLOPs; v also needs the box-filter
    # vsum).  q pre-scaled by D^-0.5; q/k shipped as fp16 transposed.
    q = (x @ np.asarray(Wq, np.float32) + np.asarray(bq, np.float32)) * SCALE
    k = x @ np.asarray(Wk, np.float32) + np.asarray(bk, np.float32)
    v = x @ np.asarray(Wv, np.float32) + np.asarray(bv, np.float32)  # [4,S,E]
    q16 = q.astype(np.float16)
    k16 = k.astype(np.float16)
    cs = np.zeros((4, S + 1, E), np.float32)
    cs[:, 1:] = np.cumsum(v, axis=1)
    vsum = cs[:, WIN:S + 1] - cs[:, 0:K]  # [4, K, E]
    # vsaug[c][h, p, cs, 0:32] = vsum[kx(p,cs)] of head h, col 32 = ones mask
    p_ar = np.arange(128)
    c_ar = np.arange(NCO)
    kx = (p_ar[:, None] % RB) + BLK * (p_ar[:, None] // RB) + RB * c_ar[None, :]
    valid = (kx < K) & (p_ar[:, None] < NP)
    kxc = np.minimum(kx, K - 1)
    in_maps = []
    for c in range(NCORES):
        b, hg = c // 2, c % 2
        sl = slice(hg * 128, (hg + 1) * 128)
        va = np.zeros((NHPC, 128, NCO, 33), np.float32)
        for h in range(NHPC):
            vh = vsum[b][:, hg * 128 + h * 32: hg * 128 + (h + 1) * 32]
            va[h, :, :, 0:32] = vh[kxc] * valid[:, :, None]
            va[h, :, :, 32] = valid.astype(np.float32)
        qp = np.zeros((128, QW), np.float16)
        qp[:, PAD:PAD + S] = q16[b, :, sl].T
        kp = np.zeros((128, QW), np.float16)
        kp[:, 0:S] = k16[b, :, sl].T
        in_maps.append({
            "qT": qp,
            "kT": kp,
            "vsaug": np.ascontiguousarray(va.astype(bfd)),
        })
    return in_maps


def kernel(x, Wq, bq, Wk, bk, Wv, bv):
    from concourse.bass_utils import run_bass_kernel_spmd

    nc = _get_nc()
    in_maps = build_in_maps(x, Wq, bq, Wk, bk, Wv, bv)
    res = run_bass_kernel_spmd(nc, in_maps, list(range(NCORES)))
    out = np.empty((4, S, E), np.float32)
    for c in range(NCORES):
        b, hg = c // 2, c % 2
        po = res.results[c]["po"]  # [S, NHPC*33]
        for h in range(NHPC):
            blk = po[:, h * 33:(h + 1) * 33]
            out[b, :, hg * 128 + h * 32: hg * 128 + (h + 1) * 32] = (
                blk[:, 0:32] / blk[:, 32:33])
    return out
